# revision 30
# baseline (speedup 1.0000x reference)
"""DeepISP Trainium2 Bass kernel — 8-core SPMD, H-sharded with halo
redundancy, staggered fold-2 row layout, bf16 matmuls with fp32 PSUM.

Sharding: core c owns output rows [64c, 64c+64). Local row l <-> global
64c - 12 + l, l in [0, 88). All full-res convs are computed per-core on
the halo-extended slice; the high-level path runs sharded down to pool2,
one AllGather replicates pool2, every core computes the tiny h3/gap/
linear tail and the 3x10 color matrix locally; the per-pixel quadratic
Tform is applied to owned rows.

Staggered fold layout: activation buffers are [128, 41 super-rows, 514]
bf16. Layer k's output super-row s holds local rows (2s+k, 2s+1+k) — the
pairing shifts by one row per conv layer. With input pairing (2s+d,
2s+1+d) and output pairing (2s+d+1, 2s+d+2), ALL six (kh x row-parity)
taps of a 3x3 conv land in exactly two full 128-wide matmuls per kw:
    acc[s] += WA_kw^T @ src[:, s,   kw:kw+512]
    acc[s] += WB_kw^T @ src[:, s+1, kw:kw+512]
i.e. 6 full-PE [128x128] matmuls per output super-row into a single
PSUM bank (vs 3 dense + 6 quarter-utilized quads for aligned pairing).

Channel layout L: p = 61*rho + c for rh channels c<61, p = 122 + 3*rho
+ (c-61) for the 3 lh (tanh) channels — relu region [0:122) and tanh
region [122:128) are contiguous, so per super-row activation is one ACT
op (tanh) + one DVE op (relu). Out-of-image edge rows are zeroed for
free via per-partition (scale, bias) columns on the activations.

low0 runs as a 36-row im2col (4 input rows x 9 taps) producing both
rows of a super-row in ONE matmul. The high-level tail avoids the DRAM
round-trip: pool1 is built directly from the folded h1 with one
SBUF->SBUF parity-realign DMA. The Tform color transform is pair-packed
(two super-rows per matmul via PE quadrants) with feats precomputed
during the hl0/h1 phase so only ~16 matmuls remain after the AllGather.
"""
import os
import sys

for _p in ("/opt/trn_rl_repo", "/root/.axon_site/_ro/trn_rl_repo"):
    if os.path.isdir(_p) and _p not in sys.path:
        sys.path.insert(0, _p)

import numpy as np
import ml_dtypes
from contextlib import ExitStack

import concourse.bass as bass
from concourse import bacc
import concourse.mybir as mybir
import concourse.tile as tile
from concourse.bass_utils import run_bass_kernel_spmd

bf16 = mybir.dt.bfloat16
f32 = mybir.dt.float32
AF = mybir.ActivationFunctionType
ALU = mybir.AluOpType
nbf = ml_dtypes.bfloat16

NCORES = 8
H = W = 512
HALO = 12          # local row 0 = global 64c-12
LR = 88
NSR = 41
SLAB = 514
BATCH = 4

# (name, row-shift d, s_lo, s_hi): out sr s = local rows (2s+d, 2s+1+d)
CONV_LAYERS = [("low0", 0, 1, 40), ("ll1", 1, 1, 39), ("ll2", 2, 1, 38),
               ("ll3", 3, 1, 37), ("ll4", 4, 1, 36), ("hl0", 5, 1, 35)]

XRUNS = [(0, 4), (4, 3), (7, 2)]          # r,g,b -> X row runs
YRUNS = [[0], [1, 4], [2, 5, 7]]          # r,g,b -> Y rows

_cached = {}


def _edge_set(d, s_lo, s_hi):
    top = [s for s in range(s_lo, s_hi + 1) if 2 * s + d <= 11]
    bot = [s for s in range(s_lo, s_hi + 1) if 2 * s + 1 + d >= 76]
    return top + bot


def _edge_cols():
    cols = {}
    n = 0
    for li, (name, d, lo, hi) in enumerate(CONV_LAYERS):
        for s in _edge_set(d, lo, hi):
            cols[(li, s)] = n
            n += 1
    return cols, n


EDGE_COLS, N_EDGE = _edge_cols()   # 36 conv edge cols; col N_EDGE = h1 slot0
NECOL = N_EDGE + 1


def _batches(lo, hi, bsz=BATCH):
    out, j = [], lo
    while j <= hi:
        out.append(list(range(j, min(j + bsz, hi + 1))))
        j += bsz
    return out


def _pL(rho, c):
    return 61 * rho + c if c < 61 else 122 + 3 * rho + (c - 61)


def _pH(rho, c):
    return 64 * rho + c


def _inv(pfun):
    inv = [None] * 128
    for rho in range(2):
        for c in range(64):
            inv[pfun(rho, c)] = (rho, c)
    return inv


INV_L = _inv(_pL)
INV_H = _inv(_pH)
RHO_L = np.array([INV_L[p][0] for p in range(128)])
RHO_H = np.array([INV_H[p][0] for p in range(128)])


# ---------------------------------------------------------------------------
# host-side weight packing
# ---------------------------------------------------------------------------

def _pack_low0(w):  # [64,3,3,3] -> [36, 128]
    out = np.zeros((36, 128), np.float32)
    for q in range(128):
        rho, cq = INV_L[q]
        for r in range(4):
            kh = r - rho
            if 0 <= kh <= 2:
                for kw in range(3):
                    out[r * 9 + kw * 3:r * 9 + kw * 3 + 3, q] = w[cq, :, kh, kw]
    return out


def _pack_stag(wf, inv_in, inv_out):  # [64o,64i,3,3] -> [128, 6*128]
    out = np.zeros((128, 6, 128), np.float32)
    for p in range(128):
        rin, cin = inv_in[p]
        for q in range(128):
            rout, cout = inv_out[q]
            for kw in range(3):
                kh = rin - rout
                if 0 <= kh <= 2:
                    out[p, 2 * kw, q] = wf[cout, cin, kh, kw]
                kh = 2 + rin - rout
                if 0 <= kh <= 2:
                    out[p, 2 * kw + 1, q] = wf[cout, cin, kh, kw]
    return out.reshape(128, 6 * 128)


def _pack_h1(w):  # [64,64,3,3] -> [128, 9*128]; u = 3*t_rel + kw
    out = np.zeros((128, 9, 128), np.float32)
    for p in range(128):
        rin, cin = INV_H[p]
        for q in range(128):
            rout, cout = INV_H[q]
            for t in range(3):
                kh = 2 * t + rin - 2 * rout
                if 0 <= kh <= 2:
                    for kw in range(3):
                        out[p, 3 * t + kw, q] = w[cout, cin, kh, kw]
    return out.reshape(128, 9 * 128)


def _pack_im2col_w(w):
    out = np.zeros((64, 9 * 64), np.float32)
    for t in range(9):
        kh, kw = divmod(t, 3)
        out[:, t * 64:(t + 1) * 64] = w[:, :, kh, kw].T
    return out


def _bias_vec(b64, pfun):
    out = np.zeros(128, np.float32)
    for rho in range(2):
        for c in range(64):
            out[pfun(rho, c)] = b64[c]
    return out


def _pack_weights(inp):
    pk = {}
    pk["w_low0"] = _pack_low0(np.asarray(inp["low0_w"]))
    stag, biases = [], []
    biases.append(_bias_vec(np.asarray(inp["low0_b"]), _pL))
    for i in range(4):
        wf = np.zeros((64, 64, 3, 3), np.float32)
        wf[:61, :61] = np.asarray(inp["ll_rh_w"])[i]
        wf[61:, 61:] = np.asarray(inp["ll_lh_w"])[i]
        stag.append(_pack_stag(wf, INV_L, INV_L))
        biases.append(_bias_vec(np.concatenate([
            np.asarray(inp["ll_rh_b"])[i], np.asarray(inp["ll_lh_b"])[i]]), _pL))
    whl0 = np.zeros((64, 64, 3, 3), np.float32)
    whl0[:, :61] = np.asarray(inp["hl0_w"])
    stag.append(_pack_stag(whl0, INV_L, INV_H))
    biases.append(_bias_vec(np.asarray(inp["hl0_b"]), _pH))
    pk["w_stag"] = np.concatenate(stag, axis=1)        # [128, 5*768]
    pk["w_h1"] = _pack_h1(np.asarray(inp["hl_w"])[0])
    biases.append(_bias_vec(np.asarray(inp["hl_b"])[0], _pH))

    bias = np.zeros((128, 9), np.float32)
    for k in range(7):
        bias[:, k] = biases[k]
    bias[0:64, 7] = np.asarray(inp["hl_b"])[1]
    bias[0:64, 8] = np.asarray(inp["hl_b"])[2]
    pk["bias"] = bias
    pk["bias_cols"] = biases                           # low0,ll1-4,hl0,h1

    pk["w_h2h3"] = np.concatenate(
        [_pack_im2col_w(np.asarray(inp["hl_w"])[1]),
         _pack_im2col_w(np.asarray(inp["hl_w"])[2])], 1)
    pk["w_lin"] = (np.asarray(inp["lin_w"]).T / 64.0).astype(np.float32)
    selL = np.zeros((30, 20), np.float32)
    for q in range(30):
        for p in range(20):
            if q % 10 == p % 10:
                selL[q, p] = 1.0
    pk["w_sel"] = selL
    cmask = np.zeros((30, 6), np.float32)
    for q in range(30):
        for n in range(6):
            if q // 10 == n % 3:
                cmask[q, n] = 1.0
    pk["cmask"] = cmask
    pmask = np.zeros((20, 6), np.float32)
    for p in range(20):
        for n in range(6):
            if p // 10 == n // 3:
                pmask[p, n] = 1.0
    pk["pmask"] = pmask
    pmask2 = np.zeros((52, 6), np.float32)
    pmask2[0:20] = pmask
    pmask2[32:52] = pmask
    pk["pmask2"] = pmask2
    pk["lin_b"] = np.asarray(inp["lin_b"]).reshape(30, 1).astype(np.float32)
    return pk


# ---------------------------------------------------------------------------
# device program
# ---------------------------------------------------------------------------

def _build_program():
    nc = bacc.Bacc("TRN2", target_bir_lowering=False, debug=False,
                   num_devices=NCORES)

    xim_in = nc.dram_tensor("x_im", [36, 40 * 512], bf16, kind="ExternalInput")
    wlow0_in = nc.dram_tensor("w_low0", [36, 128], bf16, kind="ExternalInput")
    wstag_in = nc.dram_tensor("w_stag", [128, 5 * 768], bf16,
                              kind="ExternalInput")
    wh1_in = nc.dram_tensor("w_h1", [128, 1152], bf16, kind="ExternalInput")
    wh23_in = nc.dram_tensor("w_h2h3", [64, 1152], bf16, kind="ExternalInput")
    wsel_in = nc.dram_tensor("w_sel", [30, 20], bf16, kind="ExternalInput")
    cmask_in = nc.dram_tensor("cmask", [30, 6], f32, kind="ExternalInput")
    pmask_in = nc.dram_tensor("pmask", [20, 6], f32, kind="ExternalInput")
    wlin_in = nc.dram_tensor("w_lin", [64, 30], f32, kind="ExternalInput")
    pmask2_in = nc.dram_tensor("pmask2", [52, 6], f32, kind="ExternalInput")
    bias_in = nc.dram_tensor("bias", [128, 9], f32, kind="ExternalInput")
    linb_in = nc.dram_tensor("lin_b", [30, 1], f32, kind="ExternalInput")
    esc_in = nc.dram_tensor("esc", [128, NECOL], f32, kind="ExternalInput")
    ebi_in = nc.dram_tensor("ebi", [128, NECOL], f32, kind="ExternalInput")

    out_d = nc.dram_tensor("out", [3, 64, W], f32, kind="ExternalOutput")

    cc_in = nc.dram_tensor("cc_in", [64 * 4 * 32], bf16)
    cc_gath = nc.dram_tensor("cc_gath", [NCORES * 64 * 4 * 32], bf16,
                             addr_space="Shared")

    with tile.TileContext(nc) as tc, ExitStack() as ctx:
        pers = ctx.enter_context(tc.tile_pool(name="pers", bufs=1))
        psum = ctx.enter_context(tc.tile_pool(name="psum", bufs=2, space="PSUM"))

        # ---- small persistent inputs ----
        w_low0 = pers.tile([36, 128], bf16)
        nc.sync.dma_start(w_low0[:], wlow0_in[:])
        bias_sb = pers.tile([128, 9], f32)
        nc.sync.dma_start(bias_sb[:], bias_in[:])
        esc_sb = pers.tile([128, NECOL], f32)
        nc.sync.dma_start(esc_sb[:], esc_in[:])
        ebi_sb = pers.tile([128, NECOL], f32)
        nc.sync.dma_start(ebi_sb[:], ebi_in[:])
        w_sel = pers.tile([30, 20], bf16)
        nc.sync.dma_start(w_sel[:], wsel_in[:])
        cmask_sb = pers.tile([30, 6], f32)
        nc.sync.dma_start(cmask_sb[:], cmask_in[:])
        pmask_sb = pers.tile([20, 6], f32)
        nc.sync.dma_start(pmask_sb[:], pmask_in[:])
        pmask2_sb = pers.tile([52, 6], f32)
        nc.sync.dma_start(pmask2_sb[:], pmask2_in[:])
        w_lin = pers.tile([64, 30], f32)
        nc.sync.dma_start(w_lin[:], wlin_in[:])
        linb_sb = pers.tile([30, 1], f32)
        nc.sync.dma_start(linb_sb[:], linb_in[:])

        bufA = pers.tile([128, NSR, SLAB], bf16)
        bufB = pers.tile([128, NSR, SLAB], bf16)
        xT = pers.tile([52, 16 * SLAB], bf16)
        yT = pers.tile([52, 16 * SLAB], bf16)
        featsT = pers.tile([52, 16 * SLAB], bf16)
        wt52 = pers.tile([52, 12], bf16)
        h1fold = pers.tile([128, 17, 256], bf16)
        tmpw = pers.tile([128, 17, 128], bf16)
        tmpw2 = pers.tile([64, 17, 128], bf16)
        pool1 = pers.tile([64, 17, 130], bf16)
        p2f = pers.tile([64, 34, 34], bf16)

        for buf in (bufA, bufB):
            nc.gpsimd.memset(buf[:, :, 0:1], 0.0)
            nc.gpsimd.memset(buf[:, :, 513:514], 0.0)
        nc.gpsimd.memset(xT[:], 1.0)
        nc.gpsimd.memset(yT[:], 1.0)
        nc.gpsimd.memset(wt52[:], 0.0)
        nc.gpsimd.memset(pool1[:], 0.0)
        nc.gpsimd.memset(p2f[:], 0.0)

        w_stag = pers.tile([128, 5 * 768], bf16)
        w_h1 = pers.tile([128, 1152], bf16)
        w_h2h3 = pers.tile([64, 1152], bf16)

        def act_split(dst_sr, acc, bcol, ecol):
            """identity+bias (split cols ACT/DVE), scale-masked on edges."""
            if ecol is None:
                be = bias_sb[:, bcol:bcol + 1]
                nc.scalar.activation(dst_sr[:, 0:256], acc[:, 0:256],
                                     AF.Identity, bias=be)
                nc.vector.tensor_scalar(dst_sr[:, 256:512], acc[:, 256:512],
                                        be, None, ALU.add)
            else:
                sc = esc_sb[:, ecol:ecol + 1]
                bi = ebi_sb[:, ecol:ecol + 1]
                nc.scalar.activation(dst_sr[:, 0:256], acc[:, 0:256],
                                     AF.Identity, bias=bi, scale=sc)
                nc.vector.tensor_scalar(dst_sr[:, 256:512], acc[:, 256:512],
                                        sc, bi, ALU.mult, ALU.add)

        # ---- low0 via 36-row im2col ----
        CHUNKS = [(1, 4), (5, 14), (15, 24), (25, 33), (34, 40)]
        with tc.tile_pool(name="imcp", bufs=4) as imcp:
            def chunk_dma(c0, c1):
                ns = c1 - c0 + 1
                imc = imcp.tile([36, 10 * 512], bf16, name="imc", tag="imc")
                src = bass.AP(xim_in[:].tensor, (c0 - 1) * 512,
                              [[40 * 512, 36], [1, ns * 512]])
                nc.sync.dma_start(imc[:, 0:ns * 512], src)
                return imc

            pre = [chunk_dma(*CHUNKS[0]), chunk_dma(*CHUNKS[1])]
            nc.sync.dma_start(w_stag[:], wstag_in[:])
            nc.sync.dma_start(w_h1[:], wh1_in[:])
            nc.sync.dma_start(w_h2h3[:], wh23_in[:])
            pre += [chunk_dma(*CHUNKS[k]) for k in (2, 3)]
            pre.append(None)

            for ci, (c0, c1) in enumerate(CHUNKS):
                imc = pre[ci] if pre[ci] is not None else chunk_dma(c0, c1)
                for bt in _batches(c0, c1):
                    accs = [psum.tile([128, 512], f32, name=f"acc{i}",
                                      tag=f"b{i}") for i in range(len(bt))]
                    for i, s in enumerate(bt):
                        se = s - c0
                        nc.tensor.matmul(accs[i][:], w_low0[:],
                                         imc[:, se * 512:(se + 1) * 512],
                                         start=True, stop=True)
                    for i, s in enumerate(bt):
                        act_split(bufA[:, s, 1:513], accs[i],
                                  0, EDGE_COLS.get((0, s)))

        # ---- staggered ll layers + hl0 ----
        def emit_stag(li, src, dst, kind):
            name, d, s_lo, s_hi = CONV_LAYERS[li]
            wofs = (li - 1) * 768
            wd = [w_stag[:, wofs + t * 128:wofs + (t + 1) * 128]
                  for t in range(6)]
            for bt in _batches(s_lo, s_hi):
                accs = [psum.tile([128, 512], f32, name=f"acc{i}",
                                  tag=f"b{i}") for i in range(len(bt))]
                for t in range(6):
                    kw, ab = divmod(t, 2)
                    for i, s in enumerate(bt):
                        nc.tensor.matmul(accs[i][:], wd[t],
                                         src[:, s + ab, kw:kw + 512],
                                         start=(t == 0), stop=(t == 5))
                for i, s in enumerate(bt):
                    acc, ecol = accs[i], EDGE_COLS.get((li, s))
                    if kind == "ll":
                        # PSUM partition base must be 32-aligned: tanh reads
                        # [96:128] (garbage into rh 96..121), relu then
                        # overwrites [0:122) with the correct values.
                        if ecol is None:
                            be = bias_sb[:, li:li + 1]
                            nc.scalar.activation(dst[96:128, s, 1:513],
                                                 acc[96:128, :], AF.Tanh,
                                                 bias=be[96:128])
                            nc.vector.tensor_scalar(dst[0:122, s, 1:513],
                                                    acc[0:122, :], be[0:122],
                                                    0.0, ALU.add, ALU.max)
                        else:
                            sc = esc_sb[:, ecol:ecol + 1]
                            bi = ebi_sb[:, ecol:ecol + 1]
                            nc.scalar.activation(dst[96:128, s, 1:513],
                                                 acc[96:128, :], AF.Tanh,
                                                 bias=bi[96:128],
                                                 scale=sc[96:128])
                            nc.scalar.activation(dst[0:122, s, 1:513],
                                                 acc[0:122, :], AF.Relu,
                                                 bias=bi[0:122],
                                                 scale=sc[0:122])
                    else:
                        act_split(dst[:, s, 1:513], acc, li, ecol)

        emit_stag(1, bufA, bufB, "ll")
        emit_stag(2, bufB, bufA, "ll")
        emit_stag(3, bufA, bufB, "ll")
        emit_stag(4, bufB, bufA, "ll")

        # ---- feats (from ll4 in bufA, shift 4) — overlaps hl0/h1 ----
        pstepA = bufA[:].ap[0][0]
        pstepX = xT[:].ap[0][0]
        pstepY = yT[:].ap[0][0]
        gq = [nc.sync, nc.gpsimd]
        gqi = 0
        for ci in range(3):
            p0, n = XRUNS[ci]
            for rho in range(2):
                for b in range(2):
                    soff = (bufA[:].offset + (4 + 16 * b) * SLAB
                            + (122 + 3 * rho + ci) * pstepA)
                    src = bass.AP(bufA[:].tensor, soff,
                                  [[pstepA, 1], [0, n], [1, 16 * SLAB]])
                    dst = bass.AP(
                        xT[:].tensor,
                        xT[:].offset + (32 * b + 10 * rho + p0) * pstepX,
                        [[pstepX, n], [1, 16 * SLAB]])
                    gq[gqi % 2].dma_start(dst, src)
                    gqi += 1
                    for yp in YRUNS[ci]:
                        srcy = bass.AP(bufA[:].tensor, soff,
                                       [[pstepA, 1], [1, 16 * SLAB]])
                        dsty = bass.AP(
                            yT[:].tensor,
                            yT[:].offset + (32 * b + 10 * rho + yp) * pstepY,
                            [[pstepY, 1], [1, 16 * SLAB]])
                        gq[gqi % 2].dma_start(dsty, srcy)
                        gqi += 1
        nc.vector.tensor_mul(featsT[:], xT[:], yT[:])

        emit_stag(5, bufA, bufB, "copy")

        # ---- h1 (stride-2 staggered conv from bufB, shift 5) ----
        wh1t = [w_h1[:, u * 128:(u + 1) * 128] for u in range(9)]
        for bt in _batches(0, 16):
            accs = [psum.tile([128, 256], f32, name=f"acc{i}", tag=f"b{i}")
                    for i in range(len(bt))]
            for u in range(9):
                trel, kw = divmod(u, 3)
                for i, slot in enumerate(bt):
                    sr = 2 * (slot - 1) + 3 + trel
                    nc.tensor.matmul(accs[i][:], wh1t[u],
                                     bufB[:, sr, kw:kw + 512:2],
                                     start=(u == 0), stop=(u == 8))
            for i, slot in enumerate(bt):
                sl = h1fold[:, slot, :]
                if slot == 0:
                    nc.scalar.activation(sl, accs[i][:], AF.Relu,
                                         bias=ebi_sb[:, N_EDGE:N_EDGE + 1],
                                         scale=esc_sb[:, N_EDGE:N_EDGE + 1])
                else:
                    nc.scalar.activation(sl, accs[i][:], AF.Relu,
                                         bias=bias_sb[:, 6:7])

        # ---- pool1 directly from h1fold ----
        nc.vector.tensor_max(tmpw[:], h1fold[:, :, 0:256:2],
                             h1fold[:, :, 1:256:2])
        nc.sync.dma_start(tmpw2[:], tmpw[64:128, :, :])
        nc.vector.tensor_max(pool1[:, :, 1:129], tmpw[0:64, :, :], tmpw2[:])

        with tc.tile_pool(name="hlp", bufs=1) as hlp:
            # ---- h2 via im2col ----
            imc2 = hlp.tile([64, 9 * 512], bf16)
            for t in range(9):
                kh, kw = divmod(t, 3)
                src = bass.AP(pool1[:].tensor,
                              pool1[:].offset + kh * 130 + kw,
                              [[pool1[:].ap[0][0], 64], [2 * 130, 8], [2, 64]])
                dst = imc2[:, t * 512:(t + 1) * 512]
                if t % 2 == 0:
                    nc.vector.tensor_copy(dst, src)
                else:
                    nc.scalar.activation(dst, src, AF.Copy)
            acc2 = psum.tile([64, 512], f32, name="acc0", tag="b0")
            for t in range(9):
                nc.tensor.matmul(acc2[:], w_h2h3[:, t * 64:(t + 1) * 64],
                                 imc2[:, t * 512:(t + 1) * 512],
                                 start=(t == 0), stop=(t == 8))
            h2sb = hlp.tile([64, 8, 64], bf16)
            nc.scalar.activation(h2sb[:].rearrange("p a b -> p (a b)"),
                                 acc2[:], AF.Relu, bias=bias_sb[0:64, 7:8])

            # PE warmer: keep the tensor clock from dropping to a low
            # p-state during the AllGather window (results unused)
            warm = psum.tile([128, 512], f32, name="warm", tag="b2")
            for _ in range(40):
                nc.tensor.matmul(warm[:], w_stag[:, 0:128],
                                 bufB[:, 10, 0:512], start=True, stop=True)

            # pool2 -> cc_in (DMA on gpsimd: fewest hops to the AG trigger)
            tmp2 = hlp.tile([64, 8, 32], bf16)
            nc.vector.tensor_max(tmp2[:], h2sb[:, :, 0:64:2],
                                 h2sb[:, :, 1:64:2])
            pool2 = hlp.tile([64, 4, 32], bf16)
            nc.vector.tensor_max(pool2[:], tmp2[:, 0:8:2, :],
                                 tmp2[:, 1:8:2, :])
            nc.gpsimd.dma_start(cc_in[:],
                                pool2[:].rearrange("p a b -> p (a b)"))

            # ---- AllGather pool2 ----
            with tc.tile_critical():
                cc_sem = nc.alloc_semaphore("cc_sem")
                nc.gpsimd.collective_compute(
                    "AllGather", ALU.bypass,
                    replica_groups=[list(range(NCORES))],
                    ins=[cc_in[:]], outs=[cc_gath[:]],
                ).then_inc(cc_sem)
                nc.gpsimd.wait_ge(cc_sem, 1)

            # ---- h3 tail (replicated) ----
            for q in range(NCORES):
                src = bass.AP(cc_gath[:].tensor, q * 64 * 4 * 32,
                              [[4 * 32, 64], [32, 4], [1, 32]])
                nc.sync.dma_start(p2f[:, 1 + 4 * q:5 + 4 * q, 1:33], src)
            imc3 = hlp.tile([64, 9 * 256], bf16)
            for t in range(9):
                kh, kw = divmod(t, 3)
                src = bass.AP(p2f[:].tensor, p2f[:].offset + kh * 34 + kw,
                              [[p2f[:].ap[0][0], 64], [2 * 34, 16], [2, 16]])
                dst = imc3[:, t * 256:(t + 1) * 256]
                if t % 2 == 0:
                    nc.vector.tensor_copy(dst, src)
                else:
                    nc.scalar.activation(dst, src, AF.Copy)
            acc3 = psum.tile([64, 256], f32, name="acc1", tag="b1")
            for t in range(9):
                nc.tensor.matmul(acc3[:],
                                 w_h2h3[:, 576 + t * 64:576 + (t + 1) * 64],
                                 imc3[:, t * 256:(t + 1) * 256],
                                 start=(t == 0), stop=(t == 8))
            h3sb = hlp.tile([64, 16, 16], bf16)
            nc.scalar.activation(h3sb[:].rearrange("p a b -> p (a b)"),
                                 acc3[:], AF.Relu, bias=bias_sb[0:64, 8:9])
            tmp3 = hlp.tile([64, 16, 8], bf16)
            nc.vector.tensor_max(tmp3[:], h3sb[:, :, 0:16:2],
                                 h3sb[:, :, 1:16:2])
            h3p = hlp.tile([64, 8, 8], f32)
            nc.vector.tensor_max(h3p[:], tmp3[:, 0:16:2, :],
                                 tmp3[:, 1:16:2, :])
            gsum = hlp.tile([64, 1], f32)
            nc.vector.reduce_sum(gsum[:],
                                 h3p[:].rearrange("p a b -> p (a b)"),
                                 axis=mybir.AxisListType.X)
            accW = psum.tile([30, 1], f32, name="acc2", tag="b2")
            nc.tensor.matmul(accW[:], w_lin[:], gsum[:],
                             start=True, stop=True)
            wp_sb = hlp.tile([30, 1], f32)
            nc.scalar.activation(wp_sb[:], accW[:], AF.Identity,
                                 bias=linb_sb[:])
            wpR = hlp.tile([30, 6], bf16)
            nc.vector.tensor_scalar_mul(wpR[:], cmask_sb[:], wp_sb[:])
            accM = psum.tile([52, 6], f32, name="acc3", tag="b3")
            nc.tensor.matmul(accM[0:20, :], w_sel[:], wpR[:],
                             start=True, stop=True, tile_position=(0, 0))
            nc.tensor.matmul(accM[32:52, :], w_sel[:], wpR[:],
                             start=True, stop=True, tile_position=(0, 32))
            nc.vector.tensor_tensor(wt52[0:20, 0:6], accM[0:20, :],
                                    pmask2_sb[0:20, :], ALU.mult)
            nc.vector.tensor_tensor(wt52[32:52, 6:12], accM[32:52, :],
                                    pmask2_sb[32:52, :], ALU.mult)

            # ---- Tform: pair-packed matmuls; pair p = (sr 4+p, sr 20+p) ----
            with tc.tile_pool(name="ostp", bufs=2) as ostp:
                oqs = [nc.sync, nc.gpsimd]
                for bi, bt in enumerate(_batches(0, 15)):
                    accs = [psum.tile([12, 512], f32, name=f"acc{i}",
                                      tag=f"b{i}") for i in range(len(bt))]
                    ost = ostp.tile([12, 4, 512], f32, name="ost", tag="ost")
                    for i, p in enumerate(bt):
                        nc.tensor.matmul(accs[i][:], wt52[:],
                                         featsT[:, p * SLAB + 1:p * SLAB + 513],
                                         start=True, stop=True)
                    for i, p in enumerate(bt):
                        dst = ost[:, i, :]
                        if i % 2 == 0:
                            nc.vector.tensor_copy(dst, accs[i][:])
                        else:
                            nc.scalar.activation(dst, accs[i][:], AF.Copy)
                    p0 = bt[0]
                    pstepO = ost[:].ap[0][0]
                    for half, prow in ((0, 0), (1, 6)):
                        for rho in range(2):
                            src = bass.AP(
                                ost[:].tensor,
                                ost[:].offset + (prow + 3 * rho) * pstepO,
                                [[pstepO, 3], [512, len(bt)], [1, 512]])
                            dstap = bass.AP(
                                out_d[:].tensor,
                                (32 * half + 2 * p0 + rho) * 512,
                                [[64 * 512, 3], [2 * 512, len(bt)], [1, 512]])
                            oqs[(bi + half) % 2].dma_start(dstap, src)

    nc.compile()
    return nc


# ---------------------------------------------------------------------------
# host entry
# ---------------------------------------------------------------------------

def kernel(**inputs):
    inp = {k: np.asarray(v) for k, v in inputs.items()}
    if "nc" not in _cached:
        _cached["nc"] = _build_program()
    nc = _cached["nc"]

    pk = _pack_weights(inp)
    x = np.asarray(inp["x"], np.float32)[0]

    shared = {
        "w_low0": pk["w_low0"].astype(nbf),
        "w_stag": pk["w_stag"].astype(nbf),
        "w_h1": pk["w_h1"].astype(nbf),
        "w_h2h3": pk["w_h2h3"].astype(nbf),
        "w_sel": pk["w_sel"].astype(nbf),
        "cmask": pk["cmask"],
        "pmask": pk["pmask"],
        "pmask2": pk["pmask2"],
        "w_lin": pk["w_lin"],
        "bias": pk["bias"],
        "lin_b": pk["lin_b"],
    }
    bias_cols = pk["bias_cols"]
    in_maps = []
    for c in range(NCORES):
        g0 = 64 * c - HALO
        xs = np.zeros((3, LR, SLAB), np.float32)
        lo, hi = max(0, -g0), min(LR, H - g0)
        xs[:, lo:hi, 1:513] = x[:, g0 + lo:g0 + hi, :]
        # host-built low0 im2col: p = r*9 + kw*3 + ch, sr s in 1..40
        xim = np.empty((4, 3, 3, 40, 512), np.float32)
        for r in range(4):
            for kw in range(3):
                # rows 2s-1+r for s=1..40 -> rows (1+r, 3+r, ..., 79+r)
                xim[r, kw] = xs[:, 1 + r:80 + r:2, kw:kw + 512]
        xim = xim.reshape(36, 40 * 512)

        esc = np.ones((128, NECOL), np.float32)
        ebi = np.zeros((128, NECOL), np.float32)
        for li, (name, d, s_lo, s_hi) in enumerate(CONV_LAYERS):
            rho = RHO_H if name == "hl0" else RHO_L
            bvec = bias_cols[li]
            for s in _edge_set(d, s_lo, s_hi):
                col = EDGE_COLS[(li, s)]
                g = g0 + 2 * s + d + rho
                sc = ((g >= 0) & (g < H)).astype(np.float32)
                esc[:, col] = sc
                ebi[:, col] = bvec * sc
        V = 32 * c - 2 + RHO_H
        sc = ((V >= 0) & (V < 256)).astype(np.float32)
        esc[:, N_EDGE] = sc
        ebi[:, N_EDGE] = bias_cols[6] * sc

        im = dict(shared)
        im["x_im"] = xim.astype(nbf)
        im["esc"] = esc
        im["ebi"] = ebi
        in_maps.append(im)

    res = run_bass_kernel_spmd(nc, in_maps, list(range(NCORES)))
    _cached["last_results"] = res
    out = np.concatenate([res.results[c]["out"] for c in range(NCORES)], axis=1)
    return out[None].astype(np.float32)
